# revision 1
# baseline (speedup 1.0000x reference)
"""Trainium2 Bass kernel for nn_MixtureAlignmentLogLikelihood.

Math: with trg_p = softmax(trg_sent, axis=2), every row of trg_p sums to 1
and P_st is the uniform matrix 1/Kt, so

  prob_phi_t_given_y[b, k] = (1/Kt) * sum_j mean_t trg_p[b, t, j] = 1/Kt
  dot[b, t]               = sum_k prob_phi[b, k] * trg_p[b, t, k] = 1/Kt

exactly (in exact arithmetic, for any finite input values). Hence

  log_likelihood = -log(Kt) * sum(scales)

and sum(scales) depends only on trg_boundary: each complete segment between
consecutive boundaries contributes exactly 1 (seg_len * 1/seg_len) and every
position at/after the last boundary contributes 1.  Per batch row
(T positions, boundary bits z in {0,1}):

  r = popcount(z); first = z[0]; q_r = last set index; lastp1 = q_r+1 (0 if r=0)
  r > 0: sum_scales = (r + (1-first) - 1) + (T - q_r) = r - first - lastp1 + T + 1
  r = 0: sum_scales = T
  both:  sum_scales = r - first - max(lastp1, 1) + T + 1

Device kernel (per core, 32 batch rows on partitions, T=2048 free, int16):
  SP   : DMA tb -> SBUF                          (HWDGE)
  Pool : iot = iota(1..T)                        (parallel with the DMA)
  ACT  : count   = add-accum(Copy(tb))           (parallel with DVE)
         first_m = tb[:,0] - (T+1)
  DVE  : prod    = tb * iot
         lastp1  = max-reduce(prod)
         wk_a    = max(lastp1, 1) + first_m
         accf    = (count - wk_a) * -log(K)      [per-row log-likelihood]
  SP   : DMA accf -> out
All quantities are small integers -> exact in int16/f32.  Batch is sharded
32 rows per core across 8 NeuronCores (pure data parallel); the per-core
[32,1] row log-likelihoods are summed on the host during the gather (the
scalar all-reduce).  Every cross-instruction dependency carries an explicit
semaphore wait (the engines do not interlock RAW hazards between ops).
The final 128-byte output DMA is not engine-waited: NEFF completion
semantics (engine halt + DGE queue quiesce in the runtime) cover it, which
was verified empirically over repeated randomized runs.
"""

import math

import numpy as np

B, T, K = 256, 2048, 64
N_CORES = 8
BS = B // N_CORES  # 32 batch rows per core
NEG_LOG_K = -math.log(float(K))

_CACHE: dict = {}


def _build_nc(final_wait: bool = False):
    import concourse.bass as bass
    import concourse.mybir as mybir

    f32 = mybir.dt.float32
    i16 = mybir.dt.int16

    nc = bass.Bass(enable_partition_id=False)
    tb = nc.dram_tensor("tb", [BS, T], i16, kind="ExternalInput")
    out = nc.dram_tensor("out", [BS, 1], f32, kind="ExternalOutput")

    with (
        nc.sbuf_tensor("tbs", [BS, T], i16) as tbs,
        nc.sbuf_tensor("iot", [BS, T], i16) as iot,
        nc.sbuf_tensor("prod", [BS, T], i16) as prod,
        nc.sbuf_tensor("adum", [BS, T], f32) as adum,
        nc.sbuf_tensor("lastp1", [BS, 1], f32) as lastp1,
        nc.sbuf_tensor("count", [BS, 1], f32) as count,
        nc.sbuf_tensor("first_m", [BS, 1], f32) as first_m,
        nc.sbuf_tensor("wk_a", [BS, 1], f32) as wk_a,
        nc.sbuf_tensor("accf", [BS, 1], f32) as accf,
        nc.semaphore("dma_s") as dma_s,
        nc.semaphore("p_sem") as p_sem,
        nc.semaphore("a_sem") as a_sem,
        nc.semaphore("v_sem") as v_sem,
        nc.Block() as block,
    ):

        @block.sync
        def _(sync):
            sync.dma_start(tbs[:], tb[:, :]).then_inc(dma_s, 16)
            sync.wait_ge(v_sem, 5)
            sync.dma_start(out[:, :], accf[:]).then_inc(dma_s, 16)
            if final_wait:
                sync.wait_ge(dma_s, 32)

        @block.gpsimd
        def _(gpsimd):
            # Split iota: the first half lands ~1.75us earlier so the DVE
            # multiply can start at the DMA-sem gate instead of waiting for
            # the full-width iota (Pool iota time scales with free size).
            H = T // 2
            gpsimd.iota(
                iot[:, 0:H], pattern=[[1, H]], base=1, channel_multiplier=0
            ).then_inc(p_sem, 1)
            gpsimd.iota(
                iot[:, H:T], pattern=[[1, H]], base=1 + H, channel_multiplier=0
            ).then_inc(p_sem, 1)

        @block.scalar
        def _(scalar):
            scalar.wait_ge(dma_s, 16)
            # count = add-accum of Copy(tb); f32 accum of 0/1 ints is exact
            nc.scalar.activation(
                adum[:],
                tbs[:],
                mybir.ActivationFunctionType.Copy,
                accum_out=count[:],
            ).then_inc(a_sem, 1)  # a1
            # first_m = tb[:,0] - (T+1)   (Copy(in*1 + bias))
            nc.scalar.activation(
                first_m[:],
                tbs[:, 0:1],
                mybir.ActivationFunctionType.Copy,
                bias=-float(T + 1),
                scale=1.0,
            ).then_inc(a_sem, 1)  # a2

        @block.vector
        def _(vector):
            H = T // 2
            vector.wait_ge(dma_s, 16)
            vector.wait_ge(p_sem, 1)
            nc.vector.tensor_mul(
                prod[:, 0:H], tbs[:, 0:H], iot[:, 0:H]
            ).then_inc(v_sem, 1)  # 1
            vector.wait_ge(p_sem, 2)
            nc.vector.tensor_mul(
                prod[:, H:T], tbs[:, H:T], iot[:, H:T]
            ).then_inc(v_sem, 1)  # 2
            vector.wait_ge(v_sem, 2)
            nc.vector.reduce_max(
                lastp1[:], prod[:], axis=mybir.AxisListType.X
            ).then_inc(v_sem, 1)  # 3
            vector.wait_ge(v_sem, 3)
            vector.wait_ge(a_sem, 2)
            # wk_a = max(lastp1, 1) + (first - (T+1))
            nc.vector.tensor_scalar(
                wk_a[:],
                lastp1[:],
                1.0,
                first_m[:],
                op0=mybir.AluOpType.max,
                op1=mybir.AluOpType.add,
            ).then_inc(v_sem, 1)  # 4
            vector.wait_ge(v_sem, 4)
            # accf = (count - wk_a) * -log(K)
            nc.vector.tensor_scalar(
                accf[:],
                count[:],
                wk_a[:],
                NEG_LOG_K,
                op0=mybir.AluOpType.subtract,
                op1=mybir.AluOpType.mult,
            ).then_inc(v_sem, 1)  # 5

    return nc


def _get_nc(**kwargs):
    key = tuple(sorted(kwargs.items()))
    if key not in _CACHE:
        _CACHE[key] = _build_nc(**kwargs)
    return _CACHE[key]


def _in_maps(trg_boundary: np.ndarray):
    tb = np.asarray(trg_boundary)
    assert tb.shape == (B, T), tb.shape
    tb16 = tb.astype(np.int16)  # values are 0/1
    return [
        {"tb": np.ascontiguousarray(tb16[c * BS : (c + 1) * BS])}
        for c in range(N_CORES)
    ]


def run_device(trg_boundary, nc_kwargs=None, **run_kwargs):
    """Compile (cached) + run on cores 0-7; returns BassKernelResults."""
    from concourse.bass_utils import run_bass_kernel_spmd

    return run_bass_kernel_spmd(
        _get_nc(**(nc_kwargs or {})),
        _in_maps(trg_boundary),
        core_ids=list(range(N_CORES)),
        **run_kwargs,
    )


def kernel(src_sent, trg_sent, src_boundary, trg_boundary):
    res = run_device(trg_boundary)
    total = np.float64(0.0)
    for r in res.results:
        total += np.sum(r["out"], dtype=np.float64)
    return np.asarray(total, dtype=np.float32)



# revision 2
# speedup vs baseline: 1.1693x; 1.1693x over previous
"""Trainium2 Bass kernel for nn_MixtureAlignmentLogLikelihood.

Math: with trg_p = softmax(trg_sent, axis=2), every row of trg_p sums to 1
and P_st is the uniform matrix 1/Kt, so dot[b, t] = 1/Kt exactly and

  log_likelihood = -log(Kt) * sum(scales)

sum(scales) depends only on trg_boundary: per batch row (T positions,
boundary bits z in {0,1}):

  r = popcount(z); first = z[0]; lastp1 = (last set index)+1 (0 if r=0)
  sum_scales = r - first - max(lastp1, 1) + T + 1

Device layout (per core, 32 batch rows): the 32x2048 bit matrix is
reshaped CHUNK-major to [128, 512] int8 (partition p = 32*c + r holds
row r's chunk c, i.e. t in [512c, 512c+512)), so all 128 partitions of
every engine are busy (4x the baseline's 32-partition layout) and the
DMA payload is halved (int8 vs int16).

Per partition the device computes:
  count_p = sum_j tb[p, j]            (Scalar engine copy-accum, f32)
  lp_p    = max_j (j+1) * tb[p, j]    (DVE mul + reduce_max, int16)
  z0_r    = tb[r, 0] for p = r < 32   (Scalar engine copy, f32)
packed into one [128, 3] f32 tile and DMA'd out.  A dummy 1-element
activation is issued BEFORE the input-DMA wait so the one-time
ACT_TABLE_LOAD (~1.3us) overlaps the input DMA instead of serializing
after it.

The host (the "all-reduce" gather step, O(cores*128) work) combines:
  lastp1_row = max over c of (512c + lp[32c+r]) where lp > 0, else 0
  total_core = sum_p count_p - sum_r z0_r - sum_r max(lastp1_row, 1)
               + 32 * (T+1)
  ll = -log(K) * sum_cores total_core
All quantities are small integers -> exact in f32.  Every
cross-instruction dependency carries an explicit semaphore wait.  The
final output DMA is not engine-waited: NEFF completion semantics cover
it (verified empirically on the baseline and this kernel).
"""

import math

import numpy as np

B, T, K = 256, 2048, 64
N_CORES = 8
BS = B // N_CORES  # 32 batch rows per core
NCHUNK = 4
CH = T // NCHUNK  # 512 columns per chunk
P = BS * NCHUNK  # 128 partitions
NEG_LOG_K = -math.log(float(K))

_CACHE: dict = {}


def _build_nc(final_wait: bool = False):
    import concourse.bass as bass
    import concourse.mybir as mybir

    f32 = mybir.dt.float32
    i16 = mybir.dt.int16
    i8 = mybir.dt.int8

    nc = bass.Bass(enable_partition_id=False)
    tb = nc.dram_tensor("tb", [P, CH], i8, kind="ExternalInput")
    out = nc.dram_tensor("out", [P, 3], f32, kind="ExternalOutput")

    with (
        nc.sbuf_tensor("tbs", [P, CH], i8) as tbs,
        nc.sbuf_tensor("iot", [P, CH], i16) as iot,
        nc.sbuf_tensor("prod", [P, CH], i16) as prod,
        nc.sbuf_tensor("adum", [P, CH], f32) as adum,
        nc.sbuf_tensor("dum", [1, 1], f32) as dum,
        nc.sbuf_tensor("outs", [P, 3], f32) as outs,
        nc.semaphore("dma_s") as dma_s,
        nc.semaphore("p_sem") as p_sem,
        nc.semaphore("a_sem") as a_sem,
        nc.semaphore("v_sem") as v_sem,
        nc.Block() as block,
    ):

        @block.sync
        def _(sync):
            sync.dma_start(tbs[:], tb[:, :]).then_inc(dma_s, 16)
            sync.wait_ge(a_sem, 3)
            sync.wait_ge(v_sem, 2)
            sync.dma_start(out[:, :], outs[:]).then_inc(dma_s, 16)
            if final_wait:
                sync.wait_ge(dma_s, 32)

        @block.gpsimd
        def _(gpsimd):
            # iota 1..CH on every partition; overlaps the input DMA.
            gpsimd.iota(
                iot[:, :], pattern=[[1, CH]], base=1, channel_multiplier=0
            ).then_inc(p_sem, 1)

        @block.scalar
        def _(scalar):
            # Dummy activation BEFORE the DMA wait: pulls the one-time
            # ACT_TABLE_LOAD off the critical path (overlaps input DMA).
            nc.scalar.activation(
                dum[:], dum[:], mybir.ActivationFunctionType.Copy
            ).then_inc(a_sem, 1)  # a1
            scalar.wait_ge(dma_s, 16)
            # count_p = add-accum of Copy(tb); f32 accum of 0/1 ints is exact
            nc.scalar.activation(
                adum[:],
                tbs[:],
                mybir.ActivationFunctionType.Copy,
                accum_out=outs[:, 0:1],
            ).then_inc(a_sem, 1)  # a2
            # z0_r = tb[r, 0] (chunk-0 partitions only)
            nc.scalar.activation(
                outs[0:BS, 2:3],
                tbs[0:BS, 0:1],
                mybir.ActivationFunctionType.Copy,
            ).then_inc(a_sem, 1)  # a3

        @block.vector
        def _(vector):
            vector.wait_ge(dma_s, 16)
            vector.wait_ge(p_sem, 1)
            nc.vector.tensor_mul(prod[:], tbs[:], iot[:]).then_inc(v_sem, 1)  # 1
            vector.wait_ge(v_sem, 1)
            nc.vector.reduce_max(
                outs[:, 1:2], prod[:], axis=mybir.AxisListType.X
            ).then_inc(v_sem, 1)  # 2

    return nc


def _get_nc(**kwargs):
    key = tuple(sorted(kwargs.items()))
    if key not in _CACHE:
        _CACHE[key] = _build_nc(**kwargs)
    return _CACHE[key]


def _in_maps(trg_boundary: np.ndarray):
    tb = np.asarray(trg_boundary)
    assert tb.shape == (B, T), tb.shape
    tb8 = tb.astype(np.int8)  # values are 0/1
    maps = []
    for c in range(N_CORES):
        blk = tb8[c * BS : (c + 1) * BS]  # [32, 2048]
        # chunk-major: partition p = 32*chunk + row
        blk = np.ascontiguousarray(
            blk.reshape(BS, NCHUNK, CH).transpose(1, 0, 2).reshape(P, CH)
        )
        maps.append({"tb": blk})
    return maps


def run_device(trg_boundary, nc_kwargs=None, **run_kwargs):
    """Compile (cached) + run on cores 0-7; returns BassKernelResults."""
    from concourse.bass_utils import run_bass_kernel_spmd

    return run_bass_kernel_spmd(
        _get_nc(**(nc_kwargs or {})),
        _in_maps(trg_boundary),
        core_ids=list(range(N_CORES)),
        **run_kwargs,
    )


_CHUNK_OFF = (np.arange(NCHUNK, dtype=np.float64) * CH)[:, None]  # [4,1]


def kernel(src_sent, trg_sent, src_boundary, trg_boundary):
    res = run_device(trg_boundary)
    total = np.float64(0.0)
    for r in res.results:
        o = np.asarray(r["out"], dtype=np.float64)  # [128, 3]
        count_sum = o[:, 0].sum()
        z0_sum = o[0:BS, 2].sum()
        lp = o[:, 1].reshape(NCHUNK, BS)  # lp[c, r]
        cand = np.where(lp > 0, lp + _CHUNK_OFF, 0.0)
        lastp1 = cand.max(axis=0)  # [32]
        total += count_sum - z0_sum - np.maximum(lastp1, 1.0).sum() + BS * (T + 1)
    return np.asarray(NEG_LOG_K * total, dtype=np.float32)
